# revision 5
# baseline (speedup 1.0000x reference)
"""MoE layer (top-2 of 8 experts) on 8 Trainium2 NeuronCores, expert-parallel.

v2 design (vs baseline):
- Routing is sharded: each core routes only its 1/8 token slice (fp32-exact
  split-K gate matmuls on a host-transposed x slice), computes the top-2
  renormalized combine weights for all 8 experts, and an 8-way AllToAll
  delivers each expert's cw column for all 8192 tokens to its owning core.
- Compaction uses 4 rows of 2048 tokens (capacity 576/row -> 2304 slots vs
  2560) spread on partitions {0,16,32,48}, with 2 (not 3) local_scatters
  (token id + bf16 cw).
- FFN matmuls run in bf16 (w1/w3/w2 host-cast; x gathered from a bf16 copy);
  same tensor throughput as fp32r but half the weight-stream DMA (the
  baseline was at the DMA/compute ridge re-streaming fp32 weights per chunk).
- Token-tile transposes moved off the tensor engine onto DMA XBAR
  (dma_start_transpose), freeing PSUM banks so the w2 accumulation runs as
  contiguous 32-matmul PSUM chains (no DVE accumulation adds).
- Host scatter-add combine (EP combine), as baseline.

Self-contained: hardcodes shapes for x[4,2048,1024], 8 experts, H=1024,
F=4096, top-2 with renormalized softmax weights.
"""

import os

os.environ.setdefault("JAX_PLATFORMS", "")

import numpy as np
import ml_dtypes

BF16 = ml_dtypes.bfloat16

T, H, F, E = 8192, 1024, 4096, 8
P = 128
NCORES = 8
HC = H // P                  # 8 h-blocks
FT = F // P                  # 32 f-blocks
TLOC = T // NCORES           # 1024 tokens routed per core
NTL = TLOC // P              # 8 local routing tiles
R = 4                        # compaction rows
RL = T // R                  # 2048 tokens per row
K = 576                      # per-row slot capacity (seed-0 max row load 555)
C = R * K                    # 2304 compact slots per expert
CT = C // P                  # 18 slot tiles
CHUNKS = [512, 512, 512, 512, 256]
assert sum(CHUNKS) == C

_cache: dict = {}


def _build_nc():
    import concourse.mybir as mybir
    import concourse.tile as tile
    from concourse import bacc
    from concourse.bass import IndirectOffsetOnAxis
    from concourse.masks import make_identity

    dt = mybir.dt
    Alu = mybir.AluOpType
    Act = mybir.ActivationFunctionType

    nc = bacc.Bacc("TRN2", target_bir_lowering=False, num_devices=NCORES)

    xtr_in = nc.dram_tensor("xtr", [P, HC, TLOC], dt.float32, kind="ExternalInput")
    gwt_in = nc.dram_tensor("gwt", [P, HC, E], dt.float32, kind="ExternalInput")
    xg_in = nc.dram_tensor("xg", [T, H], dt.bfloat16, kind="ExternalInput")
    w1_in = nc.dram_tensor("w1t", [FT, P, HC, P], dt.bfloat16, kind="ExternalInput")
    w3_in = nc.dram_tensor("w3t", [FT, P, HC, P], dt.bfloat16, kind="ExternalInput")
    w2_in = nc.dram_tensor("w2t", [2, 8, P, 4, 512], dt.bfloat16, kind="ExternalInput")

    y_out = nc.dram_tensor("y", [C, H], dt.float32, kind="ExternalOutput")
    idx_out = nc.dram_tensor("idx", [C], dt.int32, kind="ExternalOutput")

    with tile.TileContext(nc) as tc:
        with (
            tc.tile_pool(name="const", bufs=1) as cp,
            tc.tile_pool(name="dram", bufs=1, space="DRAM") as dp,
        ):
            ident = cp.tile([P, P], dt.float32)
            make_identity(nc, ident)
            gwt = cp.tile([P, HC, E], dt.float32)
            nc.sync.dma_start(gwt[:], gwt_in[:])

            a2a_in = dp.tile([E, TLOC], dt.float32)
            a2a_out = dp.tile([NCORES, TLOC], dt.float32)

            # ---------------- routing (local 1024 tokens) ----------------
            with (
                tc.tile_pool(name="rt", bufs=2) as rm,
                tc.tile_pool(name="ps_rt", bufs=1, space="PSUM") as pr,
            ):
                xtr = rm.tile([P, HC, TLOC], dt.float32, tag="xtr", bufs=1)
                nc.sync.dma_start(xtr[:], xtr_in[:])
                cwT = rm.tile([E, TLOC], dt.float32, tag="cwT", bufs=1)
                for i in range(NTL):
                    sl_t = slice(i * P, (i + 1) * P)
                    # gate logits in 2 split-K partials (precision: top-2/3
                    # logit gaps go down to ~3e-6; must match the fp32 ref)
                    gp0 = pr.tile([P, E], dt.float32, tag="gp0", bufs=2)
                    gp1 = pr.tile([P, E], dt.float32, tag="gp1", bufs=2)
                    for k, gp in ((0, gp0), (1, gp1)):
                        for s in range(4):
                            nc.tensor.matmul(
                                gp[:], xtr[:, 4 * k + s, sl_t], gwt[:, 4 * k + s, :],
                                start=(s == 0), stop=(s == 3),
                            )
                    lg = rm.tile([P, E], dt.float32, tag="lg")
                    nc.vector.tensor_copy(lg[:], gp0[:])
                    nc.vector.tensor_tensor(lg[:], lg[:], gp1[:], op=Alu.add)

                    mx = rm.tile([P, 8], dt.float32, tag="mx")
                    nc.vector.max(mx[:], lg[:])
                    negs = rm.tile([P, 1], dt.float32, tag="negs")
                    nc.vector.tensor_tensor(negs[:], mx[:, 0:1], mx[:, 1:2], op=Alu.add)
                    nc.vector.tensor_scalar_mul(negs[:], negs[:], -1.0)
                    sig = rm.tile([P, E], dt.float32, tag="sig")
                    nc.scalar.activation(sig[:], lg[:], Act.Sigmoid, bias=negs[:], scale=2.0)
                    msk = rm.tile([P, E], dt.float32, tag="msk")
                    nc.vector.tensor_scalar(msk[:], lg[:], mx[:, 1:2], None, op0=Alu.is_ge)
                    cw8 = rm.tile([P, E], dt.float32, tag="cw8")
                    nc.vector.tensor_tensor(cw8[:], sig[:], msk[:], op=Alu.mult)

                    ptr = pr.tile([E, P], dt.float32, tag="ptr", bufs=2)
                    nc.tensor.transpose(ptr[:], cw8[:], ident[:])
                    nc.vector.tensor_copy(cwT[:, sl_t], ptr[:])

                nc.sync.dma_start(a2a_in[:], cwT[:])
                nc.gpsimd.collective_compute(
                    "AllToAll",
                    Alu.bypass,
                    replica_groups=[list(range(NCORES))],
                    ins=[a2a_in[:].opt()],
                    outs=[a2a_out[:].opt()],
                )

            # -------- compaction: [4x2048] rows on partitions {0,16,32,48} --------
            cw128 = cp.tile([P, CT], dt.float32)
            idx_i = cp.tile([P, CT], dt.int32)
            idg_i = cp.tile([P, CT], dt.int32)
            with tc.tile_pool(name="cmp", bufs=1) as sm:
                cw64 = sm.tile([64, RL], dt.float32)
                nc.vector.memset(cw64[:], 0.0)
                rows = cw64[:].rearrange("(r s) f -> r s f", s=16)[:, 0, :]
                nc.sync.dma_start(rows, a2a_out[:].rearrange("(r c) f -> r (c f)", c=2))

                mask = sm.tile([64, RL], dt.float32)
                nc.vector.tensor_scalar(mask[:], cw64[:], 0.0, None, op0=Alu.is_gt)
                zeros = sm.tile([64, RL], dt.float32)
                nc.vector.memset(zeros[:], 0.0)
                scn = sm.tile([64, RL], dt.float32)
                nc.vector.tensor_tensor_scan(
                    scn[:], mask[:], zeros[:], 0.0, Alu.add, Alu.add
                )
                pos = sm.tile([64, RL], dt.float32)
                nc.vector.tensor_tensor(pos[:], scn[:], mask[:], op=Alu.subtract)
                inb = sm.tile([64, RL], dt.float32)
                nc.vector.tensor_scalar(inb[:], pos[:], float(K - 1), None, op0=Alu.is_le)
                sel = sm.tile([64, RL], dt.float32)
                nc.vector.tensor_tensor(sel[:], mask[:], inb[:], op=Alu.mult)
                posf = sm.tile([64, RL], dt.float32)
                nc.vector.tensor_tensor(posf[:], pos[:], sel[:], op=Alu.mult)
                selm1 = sm.tile([64, RL], dt.float32)
                nc.vector.tensor_scalar(selm1[:], sel[:], 1.0, None, op0=Alu.subtract)
                nc.vector.tensor_tensor(posf[:], posf[:], selm1[:], op=Alu.add)
                posi = sm.tile([64, RL], dt.int16)
                nc.vector.tensor_copy(posi[:], posf[:])

                iop1 = sm.tile([64, RL], dt.int32)
                nc.gpsimd.iota(iop1[:], pattern=[[1, RL]], base=1, channel_multiplier=P)
                idsp1 = sm.tile([64, RL], dt.uint16)
                nc.vector.tensor_copy(idsp1[:], iop1[:])
                cwb = sm.tile([64, RL], dt.bfloat16)
                nc.vector.tensor_copy(cwb[:], cw64[:])

                pc_id = sm.tile([64, K], dt.uint16)
                nc.gpsimd.local_scatter(pc_id[:], idsp1[:], posi[:], 64, K, RL)
                pc_cw = sm.tile([64, K], dt.uint16)
                nc.gpsimd.local_scatter(
                    pc_cw[:], cwb[:].bitcast(dt.uint16), posi[:], 64, K, RL
                )

                idf = sm.tile([64, K], dt.float32)
                nc.vector.tensor_copy(idf[:], pc_id[:])
                zt = sm.tile([64, K], dt.float32)
                nc.vector.tensor_scalar(
                    zt[:], idf[:], 0.0, 8193.0, op0=Alu.is_equal, op1=Alu.mult
                )
                nc.vector.tensor_tensor(idf[:], idf[:], zt[:], op=Alu.add)
                nc.vector.tensor_scalar(idf[:], idf[:], 1.0, None, op0=Alu.subtract)
                idgf = sm.tile([64, K], dt.float32)
                nc.vector.tensor_scalar_min(idgf[:], idf[:], float(T - 1))
                cwf = sm.tile([64, K], dt.float32)
                nc.vector.tensor_copy(cwf[:], pc_cw[:].bitcast(dt.bfloat16))

                def _rows(tile_ap):
                    return tile_ap.rearrange("(r s) f -> r s f", s=16)[:, 0, :]

                idxflat = dp.tile([C], dt.float32)
                nc.sync.dma_start(idxflat[:].rearrange("(r f) -> r f", r=R), _rows(idf[:]))
                idgflat = dp.tile([C], dt.float32)
                nc.sync.dma_start(idgflat[:].rearrange("(r f) -> r f", r=R), _rows(idgf[:]))
                cwflat = dp.tile([C], dt.float32)
                nc.sync.dma_start(cwflat[:].rearrange("(r f) -> r f", r=R), _rows(cwf[:]))

                ids128 = sm.tile([P, CT], dt.float32)
                nc.sync.dma_start(ids128[:], idxflat[:].rearrange("(j p) -> p j", p=P))
                idg128 = sm.tile([P, CT], dt.float32)
                nc.sync.dma_start(idg128[:], idgflat[:].rearrange("(j p) -> p j", p=P))
                nc.sync.dma_start(cw128[:], cwflat[:].rearrange("(j p) -> p j", p=P))

                nc.vector.tensor_copy(idx_i[:], ids128[:])
                nc.sync.dma_start(idx_out[:].rearrange("(j p) -> p j", p=P), idx_i[:])
                nc.vector.tensor_copy(idg_i[:], idg128[:])

            # ---------------- expert FFN on compact tokens ----------------
            with (
                tc.tile_pool(name="f_gx", bufs=3) as fgx,
                tc.tile_pool(name="f_xT", bufs=2) as fxt,
                tc.tile_pool(name="f_hT", bufs=1) as fht,
                tc.tile_pool(name="f_w", bufs=3) as fw,
                tc.tile_pool(name="f_misc", bufs=2) as fm,
                tc.tile_pool(name="ps_f", bufs=1, space="PSUM") as pf,
            ):
                jt0 = 0
                for tc_size in CHUNKS:
                    nt = tc_size // P
                    xT = fxt.tile([P, HC, 512], dt.bfloat16, tag="xT")
                    for jj in range(nt):
                        gx = fgx.tile([P, H], dt.bfloat16, tag="gx")
                        nc.gpsimd.indirect_dma_start(
                            out=gx[:],
                            out_offset=None,
                            in_=xg_in[:],
                            in_offset=IndirectOffsetOnAxis(
                                ap=idg_i[:, jt0 + jj : jt0 + jj + 1], axis=0
                            ),
                        )
                        nc.scalar.dma_start_transpose(
                            xT[:, :, jj * P : (jj + 1) * P], gx[:]
                        )

                    # prefetch this chunk's w2 tiles (16 x 512KB bf16)
                    w2_tiles = []
                    for hn in range(2):
                        for ftg in range(8):
                            w2t = fw.tile([P, 4, 512], dt.bfloat16, tag="w2", bufs=18)
                            (nc.sync if hn == 0 else nc.scalar).dma_start(
                                w2t[:], w2_in[hn, ftg]
                            )
                            w2_tiles.append(w2t)

                    hT = fht.tile([P, FT, 512], dt.bfloat16, tag="hT", bufs=1)
                    for ft in range(FT):
                        w1t = fw.tile([P, HC, P], dt.bfloat16, tag="w1")
                        nc.sync.dma_start(w1t[:], w1_in[ft])
                        w3t = fw.tile([P, HC, P], dt.bfloat16, tag="w3")
                        nc.scalar.dma_start(w3t[:], w3_in[ft])
                        pa = pf.tile([P, 512], dt.float32, tag="pa", bufs=2)
                        pb = pf.tile([P, 512], dt.float32, tag="pb", bufs=2)
                        for hc in range(HC):
                            nc.tensor.matmul(
                                pa[:, :tc_size], w1t[:, hc, :], xT[:, hc, :tc_size],
                                start=(hc == 0), stop=(hc == HC - 1),
                            )
                        for hc in range(HC):
                            nc.tensor.matmul(
                                pb[:, :tc_size], w3t[:, hc, :], xT[:, hc, :tc_size],
                                start=(hc == 0), stop=(hc == HC - 1),
                            )
                        sl = fm.tile([P, 512], dt.float32, tag="sl")
                        nc.scalar.activation(sl[:, :tc_size], pa[:, :tc_size], Act.Silu)
                        nc.vector.tensor_tensor(
                            hT[:, ft, :tc_size], sl[:, :tc_size], pb[:, :tc_size],
                            op=Alu.mult,
                        )

                    for hn in range(2):
                        for ts in range(nt):
                            py = pf.tile([P, 512], dt.float32, tag="py", bufs=2)
                            for ftg in range(8):
                                w2t = w2_tiles[hn * 8 + ftg]
                                for j4 in range(4):
                                    nc.tensor.matmul(
                                        py[:],
                                        hT[:, ftg * 4 + j4, ts * P : (ts + 1) * P],
                                        w2t[:, j4, :],
                                        start=(ftg == 0 and j4 == 0),
                                        stop=(ftg == 7 and j4 == 3),
                                    )
                            ysb = fm.tile([P, 512], dt.float32, tag="ysb")
                            nc.vector.tensor_scalar(
                                ysb[:], py[:],
                                cw128[:, jt0 + ts : jt0 + ts + 1], None,
                                op0=Alu.mult,
                            )
                            nc.sync.dma_start(
                                y_out[:].rearrange("(a p) h -> p a h", p=P)[
                                    :, jt0 + ts, hn * 512 : (hn + 1) * 512
                                ],
                                ysb[:],
                            )
                    jt0 += nt

    nc.finalize()
    return nc


def _prep_shared(xf, gate_w, w1, w2, w3):
    """Inputs independent of the core id (cast/transpose once)."""
    gwt = np.ascontiguousarray(
        gate_w.T.reshape(HC, P, E).transpose(1, 0, 2)
    ).astype(np.float32)
    xg = xf.astype(BF16)
    w1t, w3t, w2t = [], [], []
    for e in range(NCORES):
        w1t.append(np.ascontiguousarray(
            w1[e].reshape(HC, P, FT, P).transpose(2, 1, 0, 3)).astype(BF16))
        w3t.append(np.ascontiguousarray(
            w3[e].reshape(HC, P, FT, P).transpose(2, 1, 0, 3)).astype(BF16))
        w2t.append(np.ascontiguousarray(
            w2[e].reshape(8, 4, P, 2, 512).transpose(3, 0, 2, 1, 4)).astype(BF16))
    return gwt, xg, w1t, w3t, w2t


def _prep_core_inputs(shared, xf, e):
    gwt, xg, w1t, w3t, w2t = shared
    xs = xf[e * TLOC : (e + 1) * TLOC]              # [1024, H]
    xtr = np.ascontiguousarray(xs.T.reshape(HC, P, TLOC).transpose(1, 0, 2))
    return {
        "xtr": xtr, "gwt": gwt, "xg": xg,
        "w1t": w1t[e], "w3t": w3t[e], "w2t": w2t[e],
    }


def _run(inputs, trace=False):
    from concourse.bass_utils import run_bass_kernel_spmd

    x = np.ascontiguousarray(np.asarray(inputs["x"], dtype=np.float32))
    gate_w = np.ascontiguousarray(np.asarray(inputs["gate_w"], dtype=np.float32))
    w1 = np.ascontiguousarray(np.asarray(inputs["w1"], dtype=np.float32))
    w2 = np.ascontiguousarray(np.asarray(inputs["w2"], dtype=np.float32))
    w3 = np.ascontiguousarray(np.asarray(inputs["w3"], dtype=np.float32))
    xf = x.reshape(T, H)

    # capacity safety check (host-side routing estimate; K has margin over
    # the boundary-rounding uncertainty of this estimate)
    logits = xf @ gate_w.T
    m2 = np.sort(logits, axis=1)[:, -2:-1]
    mask = logits >= m2
    pp = mask.reshape(R, RL, E).sum(axis=1)
    if pp.max() > K:
        raise RuntimeError(
            f"per-row expert token count {pp.max()} exceeds compiled "
            f"capacity K={K}; rebuild kernel.py with a larger K"
        )

    if "nc" not in _cache:
        _cache["nc"] = _build_nc()
    nc = _cache["nc"]

    shared = _prep_shared(xf, gate_w, w1, w2, w3)
    in_maps = [_prep_core_inputs(shared, xf, e) for e in range(NCORES)]
    res = run_bass_kernel_spmd(nc, in_maps, core_ids=list(range(NCORES)), trace=trace)

    out = np.zeros((T + 1, H), dtype=np.float32)
    for e in range(NCORES):
        idx = res.results[e]["idx"]
        y = res.results[e]["y"]
        out[idx] += y
    return out[:T].reshape(x.shape), res


def kernel(**inputs) -> np.ndarray:
    out, _ = _run(inputs, trace=False)
    return out


# revision 15
# speedup vs baseline: 1.1206x; 1.1206x over previous
"""MoE layer (top-2 of 8 experts) on 8 Trainium2 NeuronCores, expert-parallel.

v2 design (vs baseline):
- Routing is sharded: each core routes only its 1/8 token slice (fp32-exact
  split-K gate matmuls on a host-transposed x slice), computes the top-2
  renormalized combine weights for all 8 experts, and an 8-way AllToAll
  delivers each expert's cw column for all 8192 tokens to its owning core.
- Compaction uses 4 rows of 2048 tokens (capacity 576/row -> 2304 slots vs
  2560) spread on partitions {0,16,32,48}, with 2 (not 3) local_scatters
  (token id + bf16 cw).
- FFN matmuls run in bf16 (w1/w3/w2 host-cast; x gathered from a bf16 copy);
  same tensor throughput as fp32r but half the weight-stream DMA (the
  baseline was at the DMA/compute ridge re-streaming fp32 weights per chunk).
- Token-tile transposes moved off the tensor engine onto DMA XBAR
  (dma_start_transpose), freeing PSUM banks so the w2 accumulation runs as
  contiguous 32-matmul PSUM chains (no DVE accumulation adds).
- Host scatter-add combine (EP combine), as baseline.

Self-contained: hardcodes shapes for x[4,2048,1024], 8 experts, H=1024,
F=4096, top-2 with renormalized softmax weights.
"""

import os

os.environ.setdefault("JAX_PLATFORMS", "")

import numpy as np
import ml_dtypes

BF16 = ml_dtypes.bfloat16

T, H, F, E = 8192, 1024, 4096, 8
P = 128
NCORES = 8
HC = H // P                  # 8 h-blocks
FT = F // P                  # 32 f-blocks
TLOC = T // NCORES           # 1024 tokens routed per core
NTL = TLOC // P              # 8 local routing tiles
R = 4                        # compaction rows
RL = T // R                  # 2048 tokens per row
K = 576                      # per-row slot capacity (seed-0 max row load 555)
C = R * K                    # 2304 compact slots per expert
CT = C // P                  # 18 slot tiles
CHUNKS = [512, 512, 512, 512, 256]
assert sum(CHUNKS) == C

_cache: dict = {}


def _build_nc():
    import concourse.mybir as mybir
    import concourse.tile as tile
    from concourse import bacc
    from concourse.bass import IndirectOffsetOnAxis
    from concourse.masks import make_identity

    dt = mybir.dt
    Alu = mybir.AluOpType
    Act = mybir.ActivationFunctionType

    nc = bacc.Bacc("TRN2", target_bir_lowering=False, num_devices=NCORES)

    xtt_in = nc.dram_tensor("xtt", [8, P, 8, HC, P], dt.float32, kind="ExternalInput")
    gwt_in = nc.dram_tensor("gwt", [P, HC, E], dt.float32, kind="ExternalInput")
    esel_in = nc.dram_tensor("esel", [P, E], dt.float32, kind="ExternalInput")
    xg_in = nc.dram_tensor("xg", [T, H], dt.bfloat16, kind="ExternalInput")
    w1_in = nc.dram_tensor("w1t", [FT, P, HC, P], dt.bfloat16, kind="ExternalInput")
    w3_in = nc.dram_tensor("w3t", [FT, P, HC, P], dt.bfloat16, kind="ExternalInput")
    w2_in = nc.dram_tensor("w2t", [2, 8, P, 4, 512], dt.bfloat16, kind="ExternalInput")

    y_out = nc.dram_tensor("y", [C, H], dt.float32, kind="ExternalOutput")
    idx_out = nc.dram_tensor("idx", [C], dt.int32, kind="ExternalOutput")

    with tile.TileContext(nc) as tc:
        with (
            tc.tile_pool(name="const", bufs=1) as cp,
            tc.tile_pool(name="dram", bufs=1, space="DRAM") as dp,
        ):
            gwt = cp.tile([P, HC, E], dt.float32)
            nc.sync.dma_start(gwt[:], gwt_in[:])
            esel = cp.tile([P, E], dt.float32)
            nc.sync.dma_start(esel[:], esel_in[:])

            cwtok = dp.tile([T], dt.float32)

            # ---------------- routing (all 8192 tokens, replicated) -------
            # batched in groups of 8 tiles to amortize DVE instruction
            # overheads (the per-tile chain is ~640 small DVE ops ~ 110us)
            NG = 8           # tiles per group
            with (
                tc.tile_pool(name="rt", bufs=3) as rm,
                tc.tile_pool(name="ps_rt", bufs=1, space="PSUM") as pr,
            ):
                cw_all = rm.tile([P, T // P], dt.float32, tag="cwall", bufs=1)
                for g in range(T // P // NG):
                    xtg = rm.tile([P, NG, HC, P], dt.float32, tag="xtt", bufs=2)
                    nc.gpsimd.dma_start(xtg[:], xtt_in[g])
                    gp0 = pr.tile([P, NG, E], dt.float32, tag="gp0", bufs=2)
                    gp1 = pr.tile([P, NG, E], dt.float32, tag="gp1", bufs=2)
                    for t in range(NG):
                        # gate logits in 2 split-K partials (precision: top-2/3
                        # logit gaps go down to ~3e-6; must match the fp32 ref)
                        for k, gp in ((0, gp0), (1, gp1)):
                            for s in range(4):
                                nc.tensor.matmul(
                                    gp[:, t, :], xtg[:, t, 4 * k + s, :],
                                    gwt[:, 4 * k + s, :],
                                    start=(s == 0), stop=(s == 3),
                                )
                    lg = rm.tile([P, NG, E], dt.float32, tag="lg")
                    nc.vector.tensor_copy(lg[:], gp0[:])
                    nc.vector.tensor_tensor(lg[:], lg[:], gp1[:], op=Alu.add)

                    mx = rm.tile([P, NG, 8], dt.float32, tag="mx")
                    for t in range(NG):
                        nc.vector.max(mx[:, t, :], lg[:, t, :])
                    # negs[p,t] = -(mx0+mx1); sig = sigmoid(2*lg - mx0 - mx1)
                    negs = rm.tile([P, NG, 1], dt.float32, tag="negs")
                    nc.vector.tensor_tensor(
                        negs[:], mx[:, :, 0:1], mx[:, :, 1:2], op=Alu.add
                    )
                    nc.vector.tensor_scalar_mul(negs[:], negs[:], -0.5)
                    arg = rm.tile([P, NG, E], dt.float32, tag="arg")
                    nc.vector.tensor_tensor(
                        arg[:], lg[:], negs[:].broadcast_to([P, NG, E]), op=Alu.add
                    )
                    sig = rm.tile([P, NG, E], dt.float32, tag="sig")
                    nc.scalar.activation(sig[:], arg[:], Act.Sigmoid, scale=2.0)
                    msk = rm.tile([P, NG, E], dt.float32, tag="msk")
                    nc.vector.tensor_tensor(
                        msk[:], lg[:], mx[:, :, 1:2].broadcast_to([P, NG, E]),
                        op=Alu.is_ge,
                    )
                    cw8 = rm.tile([P, NG, E], dt.float32, tag="cw8")
                    nc.vector.tensor_tensor(cw8[:], sig[:], msk[:], op=Alu.mult)
                    nc.vector.tensor_tensor(
                        cw8[:], cw8[:],
                        esel[:].rearrange("p (o e) -> p o e", o=1).broadcast_to([P, NG, E]),
                        op=Alu.mult,
                    )
                    nc.vector.tensor_reduce(
                        cw_all[:, g * NG : (g + 1) * NG].rearrange("p (t o) -> p t o", o=1),
                        cw8[:], axis=mybir.AxisListType.X, op=Alu.add,
                    )
                nc.sync.dma_start(cwtok[:].rearrange("(i p) -> p i", p=P), cw_all[:])

            # -------- compaction: [4x2048] rows on partitions {0,16,32,48} --------
            cw128 = cp.tile([P, CT], dt.float32)
            idx_i = cp.tile([P, CT], dt.int32)
            idg_i = cp.tile([P, CT], dt.int32)
            with tc.tile_pool(name="cmp", bufs=1) as sm:
                cw64 = sm.tile([64, RL], dt.float32)
                nc.vector.memset(cw64[:], 0.0)
                rows = cw64[:].rearrange("(r s) f -> r s f", s=16)[:, 0, :]
                nc.sync.dma_start(rows, cwtok[:].rearrange("(r f) -> r f", r=R))

                mask = sm.tile([64, RL], dt.float32)
                nc.vector.tensor_scalar(mask[:], cw64[:], 0.0, None, op0=Alu.is_gt)
                zeros = sm.tile([64, RL], dt.float32)
                nc.vector.memset(zeros[:], 0.0)
                scn = sm.tile([64, RL], dt.float32)
                nc.vector.tensor_tensor_scan(
                    scn[:], mask[:], zeros[:], 0.0, Alu.add, Alu.add
                )
                pos = sm.tile([64, RL], dt.float32)
                nc.vector.tensor_tensor(pos[:], scn[:], mask[:], op=Alu.subtract)
                inb = sm.tile([64, RL], dt.float32)
                nc.vector.tensor_scalar(inb[:], pos[:], float(K - 1), None, op0=Alu.is_le)
                sel = sm.tile([64, RL], dt.float32)
                nc.vector.tensor_tensor(sel[:], mask[:], inb[:], op=Alu.mult)
                posf = sm.tile([64, RL], dt.float32)
                nc.vector.tensor_tensor(posf[:], pos[:], sel[:], op=Alu.mult)
                selm1 = sm.tile([64, RL], dt.float32)
                nc.vector.tensor_scalar(selm1[:], sel[:], 1.0, None, op0=Alu.subtract)
                nc.vector.tensor_tensor(posf[:], posf[:], selm1[:], op=Alu.add)
                posi = sm.tile([64, RL], dt.int16)
                nc.vector.tensor_copy(posi[:], posf[:])

                iop1 = sm.tile([64, RL], dt.int32)
                nc.gpsimd.iota(iop1[:], pattern=[[1, RL]], base=1, channel_multiplier=P)
                idsp1 = sm.tile([64, RL], dt.uint16)
                nc.vector.tensor_copy(idsp1[:], iop1[:])
                cwb = sm.tile([64, RL], dt.bfloat16)
                nc.vector.tensor_copy(cwb[:], cw64[:])

                pc_id = sm.tile([64, K], dt.uint16)
                nc.gpsimd.local_scatter(pc_id[:], idsp1[:], posi[:], 64, K, RL)
                pc_cw = sm.tile([64, K], dt.uint16)
                nc.gpsimd.local_scatter(
                    pc_cw[:], cwb[:].bitcast(dt.uint16), posi[:], 64, K, RL
                )

                idf = sm.tile([64, K], dt.float32)
                nc.vector.tensor_copy(idf[:], pc_id[:])
                zt = sm.tile([64, K], dt.float32)
                nc.vector.tensor_scalar(
                    zt[:], idf[:], 0.0, 8193.0, op0=Alu.is_equal, op1=Alu.mult
                )
                nc.vector.tensor_tensor(idf[:], idf[:], zt[:], op=Alu.add)
                nc.vector.tensor_scalar(idf[:], idf[:], 1.0, None, op0=Alu.subtract)
                idgf = sm.tile([64, K], dt.float32)
                nc.vector.tensor_scalar_min(idgf[:], idf[:], float(T - 1))
                cwf = sm.tile([64, K], dt.float32)
                nc.vector.tensor_copy(cwf[:], pc_cw[:].bitcast(dt.bfloat16))

                def _rows(tile_ap):
                    return tile_ap.rearrange("(r s) f -> r s f", s=16)[:, 0, :]

                idxflat = dp.tile([C], dt.float32)
                nc.sync.dma_start(idxflat[:].rearrange("(r f) -> r f", r=R), _rows(idf[:]))
                idgflat = dp.tile([C], dt.float32)
                nc.sync.dma_start(idgflat[:].rearrange("(r f) -> r f", r=R), _rows(idgf[:]))
                cwflat = dp.tile([C], dt.float32)
                nc.sync.dma_start(cwflat[:].rearrange("(r f) -> r f", r=R), _rows(cwf[:]))

                ids128 = sm.tile([P, CT], dt.float32)
                nc.sync.dma_start(ids128[:], idxflat[:].rearrange("(j p) -> p j", p=P))
                idg128 = sm.tile([P, CT], dt.float32)
                nc.sync.dma_start(idg128[:], idgflat[:].rearrange("(j p) -> p j", p=P))
                nc.sync.dma_start(cw128[:], cwflat[:].rearrange("(j p) -> p j", p=P))

                nc.vector.tensor_copy(idx_i[:], ids128[:])
                nc.sync.dma_start(idx_out[:].rearrange("(j p) -> p j", p=P), idx_i[:])
                nc.vector.tensor_copy(idg_i[:], idg128[:])

            # ---------------- expert FFN on compact tokens ----------------
            with (
                tc.tile_pool(name="f_gx", bufs=3) as fgx,
                tc.tile_pool(name="f_xT", bufs=2) as fxt,
                tc.tile_pool(name="f_hT", bufs=1) as fht,
                tc.tile_pool(name="f_w", bufs=3) as fw,
                tc.tile_pool(name="f_misc", bufs=2) as fm,
                tc.tile_pool(name="ps_f", bufs=1, space="PSUM") as pf,
            ):
                jt0 = 0
                for tc_size in CHUNKS:
                    nt = tc_size // P
                    xT = fxt.tile([P, HC, 512], dt.bfloat16, tag="xT")
                    for jj in range(nt):
                        gx = fgx.tile([P, H], dt.bfloat16, tag="gx")
                        nc.gpsimd.indirect_dma_start(
                            out=gx[:],
                            out_offset=None,
                            in_=xg_in[:],
                            in_offset=IndirectOffsetOnAxis(
                                ap=idg_i[:, jt0 + jj : jt0 + jj + 1], axis=0
                            ),
                        )
                        nc.scalar.dma_start_transpose(
                            xT[:, :, jj * P : (jj + 1) * P], gx[:]
                        )

                    # prefetch this chunk's w2 tiles (16 x 512KB bf16)
                    w2_tiles = []
                    for hn in range(2):
                        for ftg in range(8):
                            w2t = fw.tile([P, 4, 512], dt.bfloat16, tag="w2", bufs=18)
                            (nc.sync if hn == 0 else nc.scalar).dma_start(
                                w2t[:], w2_in[hn, ftg]
                            )
                            w2_tiles.append(w2t)

                    hT = fht.tile([P, FT, 512], dt.bfloat16, tag="hT", bufs=1)
                    for ft in range(FT):
                        w1t = fw.tile([P, HC, P], dt.bfloat16, tag="w1")
                        nc.sync.dma_start(w1t[:], w1_in[ft])
                        w3t = fw.tile([P, HC, P], dt.bfloat16, tag="w3")
                        nc.scalar.dma_start(w3t[:], w3_in[ft])
                        pa = pf.tile([P, 512], dt.float32, tag="pa", bufs=2)
                        pb = pf.tile([P, 512], dt.float32, tag="pb", bufs=2)
                        for hc in range(HC):
                            nc.tensor.matmul(
                                pa[:, :tc_size], w1t[:, hc, :], xT[:, hc, :tc_size],
                                start=(hc == 0), stop=(hc == HC - 1),
                            )
                        for hc in range(HC):
                            nc.tensor.matmul(
                                pb[:, :tc_size], w3t[:, hc, :], xT[:, hc, :tc_size],
                                start=(hc == 0), stop=(hc == HC - 1),
                            )
                        sl = fm.tile([P, 512], dt.float32, tag="sl")
                        nc.scalar.activation(sl[:, :tc_size], pa[:, :tc_size], Act.Silu)
                        nc.vector.tensor_tensor(
                            hT[:, ft, :tc_size], sl[:, :tc_size], pb[:, :tc_size],
                            op=Alu.mult,
                        )

                    for hn in range(2):
                        for ts in range(nt):
                            py = pf.tile([P, 512], dt.float32, tag="py", bufs=2)
                            for ftg in range(8):
                                w2t = w2_tiles[hn * 8 + ftg]
                                for j4 in range(4):
                                    nc.tensor.matmul(
                                        py[:],
                                        hT[:, ftg * 4 + j4, ts * P : (ts + 1) * P],
                                        w2t[:, j4, :],
                                        start=(ftg == 0 and j4 == 0),
                                        stop=(ftg == 7 and j4 == 3),
                                    )
                            ysb = fm.tile([P, 512], dt.float32, tag="ysb")
                            nc.vector.tensor_scalar(
                                ysb[:], py[:],
                                cw128[:, jt0 + ts : jt0 + ts + 1], None,
                                op0=Alu.mult,
                            )
                            nc.sync.dma_start(
                                y_out[:].rearrange("(a p) h -> p a h", p=P)[
                                    :, jt0 + ts, hn * 512 : (hn + 1) * 512
                                ],
                                ysb[:],
                            )
                    jt0 += nt

    nc.finalize()
    return nc


def _prep_shared(xf, gate_w, w1, w2, w3):
    """Inputs independent of the core id (cast/transpose once)."""
    gwt = np.ascontiguousarray(
        gate_w.T.reshape(HC, P, E).transpose(1, 0, 2)
    ).astype(np.float32)
    xg = xf.astype(BF16)
    xtt = np.ascontiguousarray(
        xf.reshape(8, 8, P, HC, P).transpose(0, 4, 1, 3, 2)
    ).astype(np.float32)
    w1t, w3t, w2t = [], [], []
    for e in range(NCORES):
        w1t.append(np.ascontiguousarray(
            w1[e].reshape(HC, P, FT, P).transpose(2, 1, 0, 3)).astype(BF16))
        w3t.append(np.ascontiguousarray(
            w3[e].reshape(HC, P, FT, P).transpose(2, 1, 0, 3)).astype(BF16))
        w2t.append(np.ascontiguousarray(
            w2[e].reshape(8, 4, P, 2, 512).transpose(3, 0, 2, 1, 4)).astype(BF16))
    return gwt, xg, xtt, w1t, w3t, w2t


def _prep_core_inputs(shared, xf, e):
    gwt, xg, xtt, w1t, w3t, w2t = shared
    esel = np.zeros((P, E), dtype=np.float32)
    esel[:, e] = 1.0
    return {
        "xtt": xtt, "gwt": gwt, "esel": esel, "xg": xg,
        "w1t": w1t[e], "w3t": w3t[e], "w2t": w2t[e],
    }


def _run(inputs, trace=False):
    from concourse.bass_utils import run_bass_kernel_spmd

    x = np.ascontiguousarray(np.asarray(inputs["x"], dtype=np.float32))
    gate_w = np.ascontiguousarray(np.asarray(inputs["gate_w"], dtype=np.float32))
    w1 = np.ascontiguousarray(np.asarray(inputs["w1"], dtype=np.float32))
    w2 = np.ascontiguousarray(np.asarray(inputs["w2"], dtype=np.float32))
    w3 = np.ascontiguousarray(np.asarray(inputs["w3"], dtype=np.float32))
    xf = x.reshape(T, H)

    # capacity safety check (host-side routing estimate; K has margin over
    # the boundary-rounding uncertainty of this estimate)
    logits = xf @ gate_w.T
    m2 = np.sort(logits, axis=1)[:, -2:-1]
    mask = logits >= m2
    pp = mask.reshape(R, RL, E).sum(axis=1)
    if pp.max() > K:
        raise RuntimeError(
            f"per-row expert token count {pp.max()} exceeds compiled "
            f"capacity K={K}; rebuild kernel.py with a larger K"
        )

    if "nc" not in _cache:
        _cache["nc"] = _build_nc()
    nc = _cache["nc"]

    shared = _prep_shared(xf, gate_w, w1, w2, w3)
    in_maps = [_prep_core_inputs(shared, xf, e) for e in range(NCORES)]
    res = run_bass_kernel_spmd(nc, in_maps, core_ids=list(range(NCORES)), trace=trace)

    out = np.zeros((T + 1, H), dtype=np.float32)
    for e in range(NCORES):
        idx = res.results[e]["idx"]
        y = res.results[e]["y"]
        out[idx] += y
    return out[:T].reshape(x.shape), res


def kernel(**inputs) -> np.ndarray:
    out, _ = _run(inputs, trace=False)
    return out


# revision 16
# speedup vs baseline: 1.2793x; 1.1416x over previous
"""MoE layer (top-2 of 8 experts) on 8 Trainium2 NeuronCores, expert-parallel.

v2 design (vs baseline):
- Routing is sharded: each core routes only its 1/8 token slice (fp32-exact
  split-K gate matmuls on a host-transposed x slice), computes the top-2
  renormalized combine weights for all 8 experts, and an 8-way AllToAll
  delivers each expert's cw column for all 8192 tokens to its owning core.
- Compaction uses 4 rows of 2048 tokens (capacity 576/row -> 2304 slots vs
  2560) spread on partitions {0,16,32,48}, with 2 (not 3) local_scatters
  (token id + bf16 cw).
- FFN matmuls run in bf16 (w1/w3/w2 host-cast; x gathered from a bf16 copy);
  same tensor throughput as fp32r but half the weight-stream DMA (the
  baseline was at the DMA/compute ridge re-streaming fp32 weights per chunk).
- Token-tile transposes moved off the tensor engine onto DMA XBAR
  (dma_start_transpose), freeing PSUM banks so the w2 accumulation runs as
  contiguous 32-matmul PSUM chains (no DVE accumulation adds).
- Host scatter-add combine (EP combine), as baseline.

Self-contained: hardcodes shapes for x[4,2048,1024], 8 experts, H=1024,
F=4096, top-2 with renormalized softmax weights.
"""

import os

os.environ.setdefault("JAX_PLATFORMS", "")

import numpy as np
import ml_dtypes

BF16 = ml_dtypes.bfloat16

T, H, F, E = 8192, 1024, 4096, 8
P = 128
NCORES = 8
HC = H // P                  # 8 h-blocks
FT = F // P                  # 32 f-blocks
TLOC = T // NCORES           # 1024 tokens routed per core
NTL = TLOC // P              # 8 local routing tiles
R = 4                        # compaction rows
RL = T // R                  # 2048 tokens per row
K = 576                      # per-row slot capacity (seed-0 max row load 555)
C = R * K                    # 2304 compact slots per expert
CT = C // P                  # 18 slot tiles
CHUNKS = [512, 512, 512, 512, 256]
assert sum(CHUNKS) == C

_cache: dict = {}


def _build_nc():
    import concourse.mybir as mybir
    import concourse.tile as tile
    from concourse import bacc
    from concourse.bass import IndirectOffsetOnAxis
    from concourse.masks import make_identity

    dt = mybir.dt
    Alu = mybir.AluOpType
    Act = mybir.ActivationFunctionType

    nc = bacc.Bacc("TRN2", target_bir_lowering=False, num_devices=NCORES)

    xtt_in = nc.dram_tensor("xtt", [8, P, 8, HC, P], dt.float32, kind="ExternalInput")
    gwt_in = nc.dram_tensor("gwt", [P, HC, E], dt.float32, kind="ExternalInput")
    esel_in = nc.dram_tensor("esel", [P, E], dt.float32, kind="ExternalInput")
    xg_in = nc.dram_tensor("xg", [T, H], dt.bfloat16, kind="ExternalInput")
    w1_in = nc.dram_tensor("w1t", [FT, P, HC, P], dt.bfloat16, kind="ExternalInput")
    w3_in = nc.dram_tensor("w3t", [FT, P, HC, P], dt.bfloat16, kind="ExternalInput")
    w2_in = nc.dram_tensor("w2t", [2, 8, P, 4, 512], dt.bfloat16, kind="ExternalInput")

    y_out = nc.dram_tensor("y", [C, H], dt.float32, kind="ExternalOutput")
    idx_out = nc.dram_tensor("idx", [C], dt.int32, kind="ExternalOutput")

    with tile.TileContext(nc) as tc:
        with (
            tc.tile_pool(name="const", bufs=1) as cp,
            tc.tile_pool(name="dram", bufs=1, space="DRAM") as dp,
        ):
            gwt = cp.tile([P, HC, E], dt.float32)
            nc.sync.dma_start(gwt[:], gwt_in[:])
            esel = cp.tile([P, E], dt.float32)
            nc.sync.dma_start(esel[:], esel_in[:])

            cwtok = dp.tile([T], dt.float32)

            # ---------------- routing (all 8192 tokens, replicated) -------
            # batched in groups of 8 tiles to amortize DVE instruction
            # overheads (the per-tile chain is ~640 small DVE ops ~ 110us)
            NG = 8           # tiles per group
            with (
                tc.tile_pool(name="rt", bufs=3) as rm,
                tc.tile_pool(name="ps_rt", bufs=1, space="PSUM") as pr,
            ):
                cw_all = rm.tile([P, T // P], dt.float32, tag="cwall", bufs=1)
                for g in range(T // P // NG):
                    xtg = rm.tile([P, NG, HC, P], dt.float32, tag="xtt", bufs=3)
                    # split each group's 4.2MB across both HWDGE queues
                    nc.sync.dma_start(xtg[:, : NG // 2], xtt_in[g, :, : NG // 2])
                    nc.scalar.dma_start(xtg[:, NG // 2 :], xtt_in[g, :, NG // 2 :])
                    gp0 = pr.tile([P, NG, E], dt.float32, tag="gp0", bufs=2)
                    gp1 = pr.tile([P, NG, E], dt.float32, tag="gp1", bufs=2)
                    for t in range(NG):
                        # gate logits in 2 split-K partials (precision: top-2/3
                        # logit gaps go down to ~3e-6; must match the fp32 ref)
                        for k, gp in ((0, gp0), (1, gp1)):
                            for s in range(4):
                                nc.tensor.matmul(
                                    gp[:, t, :], xtg[:, t, 4 * k + s, :],
                                    gwt[:, 4 * k + s, :],
                                    start=(s == 0), stop=(s == 3),
                                )
                    lg = rm.tile([P, NG, E], dt.float32, tag="lg")
                    nc.vector.tensor_copy(lg[:], gp0[:])
                    nc.vector.tensor_tensor(lg[:], lg[:], gp1[:], op=Alu.add)

                    mx = rm.tile([P, NG, 8], dt.float32, tag="mx")
                    for t in range(NG):
                        nc.vector.max(mx[:, t, :], lg[:, t, :])
                    # negs[p,t] = -(mx0+mx1); sig = sigmoid(2*lg - mx0 - mx1)
                    negs = rm.tile([P, NG, 1], dt.float32, tag="negs")
                    nc.vector.tensor_tensor(
                        negs[:], mx[:, :, 0:1], mx[:, :, 1:2], op=Alu.add
                    )
                    nc.vector.tensor_scalar_mul(negs[:], negs[:], -0.5)
                    arg = rm.tile([P, NG, E], dt.float32, tag="arg")
                    nc.vector.tensor_tensor(
                        arg[:], lg[:], negs[:].broadcast_to([P, NG, E]), op=Alu.add
                    )
                    sig = rm.tile([P, NG, E], dt.float32, tag="sig")
                    nc.scalar.activation(sig[:], arg[:], Act.Sigmoid, scale=2.0)
                    msk = rm.tile([P, NG, E], dt.float32, tag="msk")
                    nc.vector.tensor_tensor(
                        msk[:], lg[:], mx[:, :, 1:2].broadcast_to([P, NG, E]),
                        op=Alu.is_ge,
                    )
                    cw8 = rm.tile([P, NG, E], dt.float32, tag="cw8")
                    nc.vector.tensor_tensor(cw8[:], sig[:], msk[:], op=Alu.mult)
                    nc.vector.tensor_tensor(
                        cw8[:], cw8[:],
                        esel[:].rearrange("p (o e) -> p o e", o=1).broadcast_to([P, NG, E]),
                        op=Alu.mult,
                    )
                    nc.vector.tensor_reduce(
                        cw_all[:, g * NG : (g + 1) * NG].rearrange("p (t o) -> p t o", o=1),
                        cw8[:], axis=mybir.AxisListType.X, op=Alu.add,
                    )
                nc.sync.dma_start(cwtok[:].rearrange("(i p) -> p i", p=P), cw_all[:])

            # -------- compaction: [4x2048] rows on partitions {0,16,32,48} --------
            cw128 = cp.tile([P, CT], dt.float32)
            idx_i = cp.tile([P, CT], dt.int32)
            idg_i = cp.tile([P, CT], dt.int32)
            with tc.tile_pool(name="cmp", bufs=1) as sm:
                cw64 = sm.tile([64, RL], dt.float32)
                nc.vector.memset(cw64[:], 0.0)
                rows = cw64[:].rearrange("(r s) f -> r s f", s=16)[:, 0, :]
                nc.sync.dma_start(rows, cwtok[:].rearrange("(r f) -> r f", r=R))

                mask = sm.tile([64, RL], dt.float32)
                nc.vector.tensor_scalar(mask[:], cw64[:], 0.0, None, op0=Alu.is_gt)
                zeros = sm.tile([64, RL], dt.float32)
                nc.vector.memset(zeros[:], 0.0)
                scn = sm.tile([64, RL], dt.float32)
                nc.vector.tensor_tensor_scan(
                    scn[:], mask[:], zeros[:], 0.0, Alu.add, Alu.add
                )
                pos = sm.tile([64, RL], dt.float32)
                nc.vector.tensor_tensor(pos[:], scn[:], mask[:], op=Alu.subtract)
                inb = sm.tile([64, RL], dt.float32)
                nc.vector.tensor_scalar(inb[:], pos[:], float(K - 1), None, op0=Alu.is_le)
                sel = sm.tile([64, RL], dt.float32)
                nc.vector.tensor_tensor(sel[:], mask[:], inb[:], op=Alu.mult)
                posf = sm.tile([64, RL], dt.float32)
                nc.vector.tensor_tensor(posf[:], pos[:], sel[:], op=Alu.mult)
                selm1 = sm.tile([64, RL], dt.float32)
                nc.vector.tensor_scalar(selm1[:], sel[:], 1.0, None, op0=Alu.subtract)
                nc.vector.tensor_tensor(posf[:], posf[:], selm1[:], op=Alu.add)
                posi = sm.tile([64, RL], dt.int16)
                nc.vector.tensor_copy(posi[:], posf[:])

                iop1 = sm.tile([64, RL], dt.int32)
                nc.gpsimd.iota(iop1[:], pattern=[[1, RL]], base=1, channel_multiplier=P)
                idsp1 = sm.tile([64, RL], dt.uint16)
                nc.vector.tensor_copy(idsp1[:], iop1[:])
                cwb = sm.tile([64, RL], dt.bfloat16)
                nc.vector.tensor_copy(cwb[:], cw64[:])

                pc_id = sm.tile([64, K], dt.uint16)
                nc.gpsimd.local_scatter(pc_id[:], idsp1[:], posi[:], 64, K, RL)
                pc_cw = sm.tile([64, K], dt.uint16)
                nc.gpsimd.local_scatter(
                    pc_cw[:], cwb[:].bitcast(dt.uint16), posi[:], 64, K, RL
                )

                idf = sm.tile([64, K], dt.float32)
                nc.vector.tensor_copy(idf[:], pc_id[:])
                zt = sm.tile([64, K], dt.float32)
                nc.vector.tensor_scalar(
                    zt[:], idf[:], 0.0, 8193.0, op0=Alu.is_equal, op1=Alu.mult
                )
                nc.vector.tensor_tensor(idf[:], idf[:], zt[:], op=Alu.add)
                nc.vector.tensor_scalar(idf[:], idf[:], 1.0, None, op0=Alu.subtract)
                idgf = sm.tile([64, K], dt.float32)
                nc.vector.tensor_scalar_min(idgf[:], idf[:], float(T - 1))
                cwf = sm.tile([64, K], dt.float32)
                nc.vector.tensor_copy(cwf[:], pc_cw[:].bitcast(dt.bfloat16))

                def _rows(tile_ap):
                    return tile_ap.rearrange("(r s) f -> r s f", s=16)[:, 0, :]

                idxflat = dp.tile([C], dt.float32)
                nc.sync.dma_start(idxflat[:].rearrange("(r f) -> r f", r=R), _rows(idf[:]))
                idgflat = dp.tile([C], dt.float32)
                nc.sync.dma_start(idgflat[:].rearrange("(r f) -> r f", r=R), _rows(idgf[:]))
                cwflat = dp.tile([C], dt.float32)
                nc.sync.dma_start(cwflat[:].rearrange("(r f) -> r f", r=R), _rows(cwf[:]))

                ids128 = sm.tile([P, CT], dt.float32)
                nc.sync.dma_start(ids128[:], idxflat[:].rearrange("(j p) -> p j", p=P))
                idg128 = sm.tile([P, CT], dt.float32)
                nc.sync.dma_start(idg128[:], idgflat[:].rearrange("(j p) -> p j", p=P))
                nc.sync.dma_start(cw128[:], cwflat[:].rearrange("(j p) -> p j", p=P))

                nc.vector.tensor_copy(idx_i[:], ids128[:])
                nc.sync.dma_start(idx_out[:].rearrange("(j p) -> p j", p=P), idx_i[:])
                nc.vector.tensor_copy(idg_i[:], idg128[:])

            # ---------------- expert FFN on compact tokens ----------------
            with (
                tc.tile_pool(name="f_gx", bufs=3) as fgx,
                tc.tile_pool(name="f_xT", bufs=2) as fxt,
                tc.tile_pool(name="f_hT", bufs=1) as fht,
                tc.tile_pool(name="f_w", bufs=3) as fw,
                tc.tile_pool(name="f_misc", bufs=2) as fm,
                tc.tile_pool(name="ps_f", bufs=1, space="PSUM") as pf,
            ):
                jt0 = 0
                for tc_size in CHUNKS:
                    nt = tc_size // P
                    xT = fxt.tile([P, HC, 512], dt.bfloat16, tag="xT")
                    for jj in range(nt):
                        gx = fgx.tile([P, H], dt.bfloat16, tag="gx")
                        nc.gpsimd.indirect_dma_start(
                            out=gx[:],
                            out_offset=None,
                            in_=xg_in[:],
                            in_offset=IndirectOffsetOnAxis(
                                ap=idg_i[:, jt0 + jj : jt0 + jj + 1], axis=0
                            ),
                        )
                        nc.scalar.dma_start_transpose(
                            xT[:, :, jj * P : (jj + 1) * P], gx[:]
                        )

                    # prefetch this chunk's w2 tiles (16 x 512KB bf16)
                    w2_tiles = []
                    for hn in range(2):
                        for ftg in range(8):
                            w2t = fw.tile([P, 4, 512], dt.bfloat16, tag="w2", bufs=18)
                            (nc.sync if hn == 0 else nc.scalar).dma_start(
                                w2t[:], w2_in[hn, ftg]
                            )
                            w2_tiles.append(w2t)

                    hT = fht.tile([P, FT, 512], dt.bfloat16, tag="hT", bufs=1)
                    for ft in range(FT):
                        w1t = fw.tile([P, HC, P], dt.bfloat16, tag="w1")
                        nc.sync.dma_start(w1t[:], w1_in[ft])
                        w3t = fw.tile([P, HC, P], dt.bfloat16, tag="w3")
                        nc.scalar.dma_start(w3t[:], w3_in[ft])
                        pa = pf.tile([P, 512], dt.float32, tag="pa", bufs=2)
                        pb = pf.tile([P, 512], dt.float32, tag="pb", bufs=2)
                        for hc in range(HC):
                            nc.tensor.matmul(
                                pa[:, :tc_size], w1t[:, hc, :], xT[:, hc, :tc_size],
                                start=(hc == 0), stop=(hc == HC - 1),
                            )
                        for hc in range(HC):
                            nc.tensor.matmul(
                                pb[:, :tc_size], w3t[:, hc, :], xT[:, hc, :tc_size],
                                start=(hc == 0), stop=(hc == HC - 1),
                            )
                        sl = fm.tile([P, 512], dt.float32, tag="sl")
                        nc.scalar.activation(sl[:, :tc_size], pa[:, :tc_size], Act.Silu)
                        nc.vector.tensor_tensor(
                            hT[:, ft, :tc_size], sl[:, :tc_size], pb[:, :tc_size],
                            op=Alu.mult,
                        )

                    for hn in range(2):
                        for ts in range(nt):
                            py = pf.tile([P, 512], dt.float32, tag="py", bufs=2)
                            for ftg in range(8):
                                w2t = w2_tiles[hn * 8 + ftg]
                                for j4 in range(4):
                                    nc.tensor.matmul(
                                        py[:],
                                        hT[:, ftg * 4 + j4, ts * P : (ts + 1) * P],
                                        w2t[:, j4, :],
                                        start=(ftg == 0 and j4 == 0),
                                        stop=(ftg == 7 and j4 == 3),
                                    )
                            ysb = fm.tile([P, 512], dt.float32, tag="ysb")
                            nc.vector.tensor_scalar(
                                ysb[:], py[:],
                                cw128[:, jt0 + ts : jt0 + ts + 1], None,
                                op0=Alu.mult,
                            )
                            nc.sync.dma_start(
                                y_out[:].rearrange("(a p) h -> p a h", p=P)[
                                    :, jt0 + ts, hn * 512 : (hn + 1) * 512
                                ],
                                ysb[:],
                            )
                    jt0 += nt

    nc.finalize()
    return nc


def _prep_shared(xf, gate_w, w1, w2, w3):
    """Inputs independent of the core id (cast/transpose once)."""
    gwt = np.ascontiguousarray(
        gate_w.T.reshape(HC, P, E).transpose(1, 0, 2)
    ).astype(np.float32)
    xg = xf.astype(BF16)
    xtt = np.ascontiguousarray(
        xf.reshape(8, 8, P, HC, P).transpose(0, 4, 1, 3, 2)
    ).astype(np.float32)
    w1t, w3t, w2t = [], [], []
    for e in range(NCORES):
        w1t.append(np.ascontiguousarray(
            w1[e].reshape(HC, P, FT, P).transpose(2, 1, 0, 3)).astype(BF16))
        w3t.append(np.ascontiguousarray(
            w3[e].reshape(HC, P, FT, P).transpose(2, 1, 0, 3)).astype(BF16))
        w2t.append(np.ascontiguousarray(
            w2[e].reshape(8, 4, P, 2, 512).transpose(3, 0, 2, 1, 4)).astype(BF16))
    return gwt, xg, xtt, w1t, w3t, w2t


def _prep_core_inputs(shared, xf, e):
    gwt, xg, xtt, w1t, w3t, w2t = shared
    esel = np.zeros((P, E), dtype=np.float32)
    esel[:, e] = 1.0
    return {
        "xtt": xtt, "gwt": gwt, "esel": esel, "xg": xg,
        "w1t": w1t[e], "w3t": w3t[e], "w2t": w2t[e],
    }


def _run(inputs, trace=False):
    from concourse.bass_utils import run_bass_kernel_spmd

    x = np.ascontiguousarray(np.asarray(inputs["x"], dtype=np.float32))
    gate_w = np.ascontiguousarray(np.asarray(inputs["gate_w"], dtype=np.float32))
    w1 = np.ascontiguousarray(np.asarray(inputs["w1"], dtype=np.float32))
    w2 = np.ascontiguousarray(np.asarray(inputs["w2"], dtype=np.float32))
    w3 = np.ascontiguousarray(np.asarray(inputs["w3"], dtype=np.float32))
    xf = x.reshape(T, H)

    # capacity safety check (host-side routing estimate; K has margin over
    # the boundary-rounding uncertainty of this estimate)
    logits = xf @ gate_w.T
    m2 = np.sort(logits, axis=1)[:, -2:-1]
    mask = logits >= m2
    pp = mask.reshape(R, RL, E).sum(axis=1)
    if pp.max() > K:
        raise RuntimeError(
            f"per-row expert token count {pp.max()} exceeds compiled "
            f"capacity K={K}; rebuild kernel.py with a larger K"
        )

    if "nc" not in _cache:
        _cache["nc"] = _build_nc()
    nc = _cache["nc"]

    shared = _prep_shared(xf, gate_w, w1, w2, w3)
    in_maps = [_prep_core_inputs(shared, xf, e) for e in range(NCORES)]
    res = run_bass_kernel_spmd(nc, in_maps, core_ids=list(range(NCORES)), trace=trace)

    out = np.zeros((T + 1, H), dtype=np.float32)
    for e in range(NCORES):
        idx = res.results[e]["idx"]
        y = res.results[e]["y"]
        out[idx] += y
    return out[:T].reshape(x.shape), res


def kernel(**inputs) -> np.ndarray:
    out, _ = _run(inputs, trace=False)
    return out
